# revision 1
# baseline (speedup 1.0000x reference)
"""Cross-attention kernel for Trainium2, 8-core tensor-parallel over heads.

Problem (fixed shapes, fp32):
    patch_embed [2, 2048, 1024], pixel_embed [2, 2048, 1024]
    Wq/Wk/Wv [1024, 1024], Wo [1024, 1024], bo [1024]
    16 heads x 64 dim_head, softmax cross-attention, out [2, 2048, 1024].

Sharding: core c handles batch b = c // 4 and head-group g = c % 4
(4 heads = 256 inner cols). Each core computes a partial output
(its heads' contribution to out @ Wo) in bf16; host sums the 4 partials
per batch and adds the bias.

Per-core device program (all matmuls bf16 with fp32 PSUM accumulation):
    inputs shipped bf16, loaded natural [seq, d] in 512-row chunks (big
    contiguous DMA), transposed on the PE (128x128 identity-matmul tiles)
    kT = Wk_g^T @ pixelT      [256, m]
    v  = pixelT^T @ Wv_g (+ ones col)  [m, 4, 65]
    qT = Wq_g^T @ patchT      [256, n]
    per (pair, 1024-query block, key tile, head):
        sT = kT_h^T @ qT_h    (K=64, 2x512 free, row-group packed)
        eT = exp(scale * sT)  (ACT, [128, 1024] psum->sbuf bf16)
        oT[65] += v_aug^T @ eT  (row 64 accumulates Z)
    oT_n = oT[0:64] * (1/Z)   (DVE recip + gpsimd bcast + DVE mul, bf16)
    y_partial = oT_n^T @ Wo_g [n, 1024]  (bf16) -> DRAM

Emission is interleaved (single tile/PSUM pool scope) so attention on
query block 0 streams while pixel chunks 1-3 are still being loaded and
projected; patch chunks 2-3 and the qb0 output projection slot into the
ACT-bound stretches of later attention groups.
"""

import numpy as np

HEADS = 16
DH = 64
B = 2
N = 2048          # query seq len
M = 2048          # key seq len
D = 1024
N_CORES = 8
HPC = 4           # heads per core
C = HPC * DH      # 256 inner cols per core
SCALE = DH ** -0.5
P = 128
FREE = 512        # fp32 matmul moving free dim (one PSUM bank)
KT_D = D // P     # 8 contraction tiles for projections
NCH = N // FREE   # 4 input chunks of 512 rows
JT = M // P       # 16 key tiles
QB = 1024         # query block for attention/output phases

_cache = {}


def _build_nc():
    import concourse.bacc as bacc
    import concourse.mybir as mybir
    import concourse.tile as tile
    from concourse.masks import make_identity

    F32 = mybir.dt.float32
    F32R = mybir.dt.float32r
    BF16 = mybir.dt.bfloat16
    EXP = mybir.ActivationFunctionType.Exp

    nc = bacc.Bacc("TRN2", target_bir_lowering=False, debug=False,
                   num_devices=N_CORES)

    pe = nc.dram_tensor("pe", [N, D], BF16, kind="ExternalInput")
    xe = nc.dram_tensor("xe", [M, D], BF16, kind="ExternalInput")
    wq = nc.dram_tensor("wq", [D, C], BF16, kind="ExternalInput")
    wk = nc.dram_tensor("wk", [D, C], BF16, kind="ExternalInput")
    wv = nc.dram_tensor("wv", [D, C], BF16, kind="ExternalInput")
    wo = nc.dram_tensor("wo", [C, D], BF16, kind="ExternalInput")
    yp = nc.dram_tensor("yp", [N, D], BF16, kind="ExternalOutput")

    # chunked views: 512 seq rows -> [128 part, 4 subtiles, 1024]
    pe_c = pe.ap().rearrange("(ch s p) d -> ch p s d", p=P, s=4)
    xe_c = xe.ap().rearrange("(ch s p) d -> ch p s d", p=P, s=4)
    wq_t = wq.ap().rearrange("(ko ki) c -> ki ko c", ki=P)   # [128,8,256]
    wk_t = wk.ap().rearrange("(ko ki) c -> ki ko c", ki=P)
    wv_t = wv.ap().rearrange("(ko ki) c -> ki ko c", ki=P)
    wo_t = wo.ap().rearrange("(ko ki) n -> ki ko n", ki=P)   # [128,2,1024]
    yp_c = yp.ap().rearrange("(qc qt p) d -> qc p qt d", p=P, qt=8)

    with tile.TileContext(nc) as tc:
        with (
            tc.tile_pool(name="const", bufs=1) as const,
            tc.tile_pool(name="wpool", bufs=1) as wpool,
            tc.tile_pool(name="natp", bufs=4) as natp,
            tc.tile_pool(name="xTp", bufs=2) as xTp,
            tc.tile_pool(name="eT", bufs=6) as epool,
            tc.tile_pool(name="rzp", bufs=2) as rzp,
            tc.tile_pool(name="rzbp", bufs=2) as rzbp,
            tc.tile_pool(name="yst", bufs=2) as yst,
            tc.tile_pool(name="mm", bufs=2, space="PSUM") as mmp,
            tc.tile_pool(name="pacc", bufs=2, space="PSUM") as pacc,
        ):
            ident = const.tile([P, P], BF16)
            make_identity(nc, ident)

            wk_r = wpool.tile([P, KT_D, C], BF16, name="wk_r")
            wv_r = wpool.tile([P, KT_D, C], BF16, name="wv_r")
            wq_r = wpool.tile([P, KT_D, C], BF16, name="wq_r")
            wo_r = wpool.tile([P, 2, D], BF16, name="wo_r")

            # per-512-chunk tiles so attention deps are chunk-granular
            kTc = [wpool.tile([P, 2, FREE], BF16, name=f"kTc{i}")
                   for i in range(NCH)]
            qTc = [wpool.tile([P, 2, FREE], BF16, name=f"qTc{i}")
                   for i in range(NCH)]
            oTq = [wpool.tile([P, 2, QB], BF16, name=f"oTq{i}")
                   for i in range(N // QB)]
            vc = [wpool.tile([P, 4, HPC, DH + 1], BF16, name=f"vc{i}")
                  for i in range(NCH)]
            for i in range(NCH):
                nc.vector.memset(vc[i][:, :, :, DH], 1.0)

            # ---------------- front-end helpers --------------------------
            def load_w(dram_ap, shape, dst):
                stage = natp.tile([P, 4, D], BF16, tag="nat", name="wstg")
                st = stage[:].rearrange("p s d -> p (s d)")[
                    :, 0:int(np.prod(shape[1:]))].rearrange(
                    "p (a b) -> p a b", a=shape[1])
                nc.sync.dma_start(out=st, in_=dram_ap)
                nc.vector.tensor_copy(dst[:], st)

            def trans_chunk(src_c, ch):
                nat = natp.tile([P, 4, D], BF16, tag="nat", name="nat")
                nc.sync.dma_start(out=nat[:], in_=src_c[ch])
                xT = xTp.tile([P, KT_D, FREE], BF16, tag="xT", name="xT")
                for k2 in range(KT_D // 2):
                    pt = mmp.tile([P, 2 * FREE], BF16, tag="mm", name="pt")
                    for k in range(2):
                        kt = k2 * 2 + k
                        for s in range(4):
                            nc.tensor.transpose(
                                pt[:, k * FREE + s * P:
                                   k * FREE + (s + 1) * P],
                                nat[:, s, kt * P:(kt + 1) * P], ident[:])
                    nc.vector.tensor_copy(
                        xT[:, k2 * 2:(k2 + 1) * 2, :],
                        pt[:].rearrange("p (k q) -> p k q", k=2))
                return xT

            def proj_kq(w_r, dstT, xT):
                pq = mmp.tile([P, 2 * FREE], F32, tag="mm", name="pq")
                for mt in range(2):
                    for kt in range(KT_D):
                        nc.tensor.matmul(
                            pq[:, mt * FREE:(mt + 1) * FREE],
                            w_r[:, kt, mt * P:(mt + 1) * P],
                            xT[:, kt, :],
                            start=(kt == 0), stop=(kt == KT_D - 1))
                nc.vector.tensor_copy(
                    dstT[:], pq[:].rearrange("p (mt q) -> p mt q", mt=2))

            def proj_v(ch, xT):
                pv = mmp.tile([P, 4 * C], F32, tag="mm", name="pv")
                for s in range(4):
                    for kt in range(KT_D):
                        nc.tensor.matmul(
                            pv[:, s * C:(s + 1) * C],
                            xT[:, kt, s * P:(s + 1) * P],
                            wv_r[:, kt, :],
                            start=(kt == 0), stop=(kt == KT_D - 1))
                nc.vector.tensor_copy(
                    vc[ch][:, :, :, 0:DH],
                    pv[:].rearrange("p (s h e) -> p s h e", s=4, h=HPC))

            # ---------------- attention helpers ---------------------------
            def attn_group(qb, pair, po, jts):
                for jt in jts:
                    kch, jl = divmod(jt, 4)
                    for hh in range(2):
                        h = pair * 2 + hh
                        pst = mmp.tile([P, QB], F32, tag="mm", name="pst")
                        for qh in range(QB // FREE):
                            qch = qb * 2 + qh
                            nc.tensor.matmul(
                                pst[:, qh * FREE:(qh + 1) * FREE],
                                kTc[kch][hh * DH:(hh + 1) * DH, pair,
                                         jl * P:(jl + 1) * P],
                                qTc[qch][hh * DH:(hh + 1) * DH, pair, :],
                                start=True, stop=True,
                                tile_position=(hh * DH, 0))
                        eT = epool.tile([P, QB], BF16, tag="eT")
                        nc.scalar.activation(eT[:], pst[:], EXP,
                                             scale=SCALE)
                        for qh in range(QB // FREE):
                            nc.tensor.matmul(
                                po[hh][:, qh * FREE:(qh + 1) * FREE],
                                vc[kch][:, jl, h, :],
                                eT[:, qh * FREE:(qh + 1) * FREE],
                                start=(jt == 0), stop=(jt == JT - 1))

            def attn_norm(qb, pair, po, qhs=(0, 1)):
                # copy PSUM accumulators to SBUF first so the po banks
                # free up for the next pair; normalize from the copies.
                for hh in range(2):
                    zc = rzp.tile([DH + 1, QB], F32, tag="zc",
                                  name=f"zc{qb}{pair}{hh}")
                    nc.vector.tensor_copy(zc[:], po[hh][:])
                    for qh in qhs:
                        sl = slice(qh * FREE, (qh + 1) * FREE)
                        rz = rzp.tile([1, FREE], F32, tag="rz")
                        nc.vector.reciprocal(rz[:], zc[DH:DH + 1, sl])
                        rzb = rzbp.tile([DH, FREE], F32, tag="rzb")
                        nc.gpsimd.partition_broadcast(rzb[:], rz[:])
                        nc.vector.tensor_mul(
                            oTq[qb][hh * DH:(hh + 1) * DH, pair, sl],
                            zc[0:DH, sl], rzb[:])

            def new_po():
                return [pacc.tile([DH + 1, QB], F32, tag="po",
                                  name=f"po{hh}") for hh in range(2)]

            def out_proj_tiles(qb, ys, qrange):
                for qt8 in qrange:
                    py = mmp.tile([P, D], F32, tag="mm", name="py")
                    for nk in range(D // FREE):
                        for ct in range(2):
                            nc.tensor.matmul(
                                py[:, nk * FREE:(nk + 1) * FREE],
                                oTq[qb][:, ct, qt8 * P:(qt8 + 1) * P],
                                wo_r[:, ct, nk * FREE:(nk + 1) * FREE],
                                start=(ct == 0), stop=(ct == 1))
                    nc.vector.tensor_copy(ys[:, qt8, :], py[:])

            # ======================= emission =============================
            # patch chunks 0-1 (query block 0), weights
            paT0 = trans_chunk(pe_c, 0)
            load_w(wq_t, [P, KT_D, C], wq_r)
            proj_kq(wq_r, qTc[0], paT0)
            paT1 = trans_chunk(pe_c, 1)
            proj_kq(wq_r, qTc[1], paT1)
            load_w(wk_t, [P, KT_D, C], wk_r)
            load_w(wv_t, [P, KT_D, C], wv_r)

            # pixel chunks interleaved with attention qb0/pair0
            po00 = new_po()
            for ch in range(NCH):
                xT = trans_chunk(xe_c, ch)
                proj_kq(wk_r, kTc[ch], xT)
                proj_v(ch, xT)
                attn_group(0, 0, po00, range(4 * ch, 4 * ch + 4))
            attn_norm(0, 0, po00)

            # qb0/pair1 with patch chunks 2-3 and wo slotted in
            po01 = new_po()
            attn_group(0, 1, po01, range(0, 4))
            paT2 = trans_chunk(pe_c, 2)
            proj_kq(wq_r, qTc[2], paT2)
            attn_group(0, 1, po01, range(4, 8))
            paT3 = trans_chunk(pe_c, 3)
            proj_kq(wq_r, qTc[3], paT3)
            load_w(wo_t, [P, 2, D], wo_r)
            attn_group(0, 1, po01, range(8, 16))
            attn_norm(0, 1, po01)

            # qb1/pair0 with qb0 output projection slotted in (late enough
            # that the qb0/pair1 normalization chain has finished)
            ys0 = yst.tile([P, 8, D], BF16, tag="ys", name="ys0")
            po10 = new_po()
            attn_group(1, 0, po10, range(0, 8))
            out_proj_tiles(0, ys0, range(0, 4))
            attn_group(1, 0, po10, range(8, 12))
            out_proj_tiles(0, ys0, range(4, 8))
            attn_group(1, 0, po10, range(12, 16))
            attn_norm(1, 0, po10)
            nc.sync.dma_start(out=yp_c[0], in_=ys0[:])

            # qb1/pair1, then qb1 output projection; the tail normalization
            # is split by query half so the projection overlaps it
            po11 = new_po()
            attn_group(1, 1, po11, range(0, 16))
            zcs = []
            for hh in range(2):
                zc = rzp.tile([DH + 1, QB], F32, tag="zc", name=f"zct{hh}")
                nc.vector.tensor_copy(zc[:], po11[hh][:])
                zcs.append(zc)
            ys1 = yst.tile([P, 8, D], BF16, tag="ys", name="ys1")
            for qh in range(2):
                sl = slice(qh * FREE, (qh + 1) * FREE)
                for hh in range(2):
                    rz = rzp.tile([1, FREE], F32, tag="rz")
                    nc.vector.reciprocal(rz[:], zcs[hh][DH:DH + 1, sl])
                    rzb = rzbp.tile([DH, FREE], F32, tag="rzb")
                    nc.gpsimd.partition_broadcast(rzb[:], rz[:])
                    nc.vector.tensor_mul(
                        oTq[1][hh * DH:(hh + 1) * DH, 1, sl],
                        zcs[hh][0:DH, sl], rzb[:])
                out_proj_tiles(1, ys1, range(qh * 4, qh * 4 + 4))
            nc.sync.dma_start(out=yp_c[1], in_=ys1[:])

    nc.compile()
    return nc


def get_nc():
    if "nc" not in _cache:
        _cache["nc"] = _build_nc()
    return _cache["nc"]


def make_core_inputs(patch_embed, pixel_embed, Wq, Wk, Wv, Wo, c):
    import ml_dtypes

    bf16 = ml_dtypes.bfloat16
    b, g = divmod(c, HPC)
    sl = slice(g * C, (g + 1) * C)
    return {
        "pe": np.ascontiguousarray(np.asarray(patch_embed[b]).astype(bf16)),
        "xe": np.ascontiguousarray(np.asarray(pixel_embed[b]).astype(bf16)),
        "wq": np.ascontiguousarray(np.asarray(Wq[:, sl]).astype(bf16)),
        "wk": np.ascontiguousarray(np.asarray(Wk[:, sl]).astype(bf16)),
        "wv": np.ascontiguousarray(np.asarray(Wv[:, sl]).astype(bf16)),
        "wo": np.ascontiguousarray(np.asarray(Wo[sl, :]).astype(bf16)),
    }


def kernel(patch_embed, pixel_embed, Wq, Wk, Wv, Wo, bo):
    from concourse.bass_utils import run_bass_kernel_spmd

    nc = get_nc()
    in_maps = [make_core_inputs(patch_embed, pixel_embed, Wq, Wk, Wv, Wo, c)
               for c in range(N_CORES)]
    res = run_bass_kernel_spmd(nc, in_maps, core_ids=list(range(N_CORES)))
    out = np.empty((B, N, D), dtype=np.float32)
    for b in range(B):
        acc = res.results[b * HPC + 0]["yp"].astype(np.float32)
        for g in range(1, HPC):
            acc = acc + res.results[b * HPC + g]["yp"].astype(np.float32)
        out[b] = acc + np.asarray(bo, dtype=np.float32)[None, :]
    return out



# revision 32
# speedup vs baseline: 1.1886x; 1.1886x over previous
"""Cross-attention kernel for Trainium2, 8-core tensor-parallel over heads.

Problem (fixed shapes, fp32):
    patch_embed [2, 2048, 1024], pixel_embed [2, 2048, 1024]
    Wq/Wk/Wv [1024, 1024], Wo [1024, 1024], bo [1024]
    16 heads x 64 dim_head, softmax cross-attention, out [2, 2048, 1024].

Sharding: core c handles batch b = c // 4 and head-group g = c % 4
(4 heads = 256 inner cols). Each core computes a partial output
(its heads' contribution to out @ Wo) in bf16; host sums the 4 partials
per batch and adds the bias.

Per-core device program (all matmuls bf16, fp32 PSUM):
  - inputs land TRANSPOSED in SBUF via XBAR DMA-transpose (no PE/DVE
    transpose cost), except patch chunk 0 which is loaded natural and
    PE-transposed to cut the pipeline-fill latency.
  - kT/qT = W^T @ xT  [ (hh,dh)=128 part, pair, seq ];  v natural.
  - attention runs in (pair, qb=512-query) groups over 16 key tiles;
    logits stream through a ring of two [128,1536] fp32 PSUM tiles so
    every ACT exp instruction is 1536 wide (amortizes ACT fixed cost).
  - attn@v uses out[q,dh] orientation: lhsT = exp-tile slice [128k,128q],
    rhs = v_aug [128k, 65] (65th col = ones -> softmax denominator Z),
    so each matmul streams only 65 free elements instead of 512.
  - normalization is a per-partition reciprocal + tensor_scalar_mul
    (Z sits on the free axis), split across DVE and Pool engines.
  - normalized o [q, (hh,dh)] is PE-transposed back to [(hh,dh), q]
    for the output projection; projections/out-proj are interleaved
    into the ACT-bound attention phase as filler work on a static
    schedule so the PE never idles.
"""

import numpy as np
from collections import deque

HEADS = 16
DH = 64
B = 2
N = 2048          # query seq len
M = 2048          # key seq len
D = 1024
N_CORES = 8
HPC = 4           # heads per core
C = HPC * DH      # 256 inner cols per core
SCALE = DH ** -0.5
P = 128
CH = 512          # input chunk span (seq rows)
NCH = N // CH     # 4 chunks
KT_D = D // P     # 8 contraction tiles for projections
JT = M // P       # 16 key tiles
RW = 1536         # exp ring width (3 x 512 chunks)
QB = 512          # queries per attention group
NQB = N // QB     # 4 query groups
QBLK = QB // P    # 4 128-query blocks per group

_cache = {}


def _build_nc():
    import concourse.bacc as bacc
    import concourse.mybir as mybir
    import concourse.tile as tile
    from concourse.masks import make_identity

    F32 = mybir.dt.float32
    BF16 = mybir.dt.bfloat16
    EXP = mybir.ActivationFunctionType.Exp

    nc = bacc.Bacc("TRN2", target_bir_lowering=False, debug=False,
                   num_devices=N_CORES)

    pe = nc.dram_tensor("pe", [N, D], BF16, kind="ExternalInput")
    xe = nc.dram_tensor("xe", [M, D], BF16, kind="ExternalInput")
    wq = nc.dram_tensor("wq", [D, C], BF16, kind="ExternalInput")
    wk = nc.dram_tensor("wk", [D, C], BF16, kind="ExternalInput")
    wv = nc.dram_tensor("wv", [D, C], BF16, kind="ExternalInput")
    wo = nc.dram_tensor("wo", [C, D], BF16, kind="ExternalInput")
    yp = nc.dram_tensor("yp", [N, D], BF16, kind="ExternalOutput")

    pe_nat = pe.ap().rearrange("(ch s p) d -> ch p s d", p=P, s=4)
    wq_v, wk_v, wv_v = (w.ap().rearrange("(ko ki) c -> ki ko c", ki=P)
                        for w in (wq, wk, wv))
    wo_t = wo.ap().rearrange("(ko ki) n -> ki ko n", ki=P)
    yp_r = yp.ap().rearrange("(qq p) d -> qq p d", p=P)   # 16 x [128,1024]

    with tile.TileContext(nc) as tc:
        with (
            tc.tile_pool(name="const", bufs=1) as const,
            tc.tile_pool(name="wpool", bufs=1) as wpool,
            tc.tile_pool(name="natp", bufs=1) as natp,
            tc.tile_pool(name="eTp", bufs=4) as eTp,
            tc.tile_pool(name="obp", bufs=2) as obp,
            tc.tile_pool(name="rzp", bufs=2) as rzp,
            tc.tile_pool(name="ysp", bufs=3) as ysp,
            tc.tile_pool(name="ring", bufs=1, space="PSUM") as ringp,
            tc.tile_pool(name="pop", bufs=1, space="PSUM") as pop,
            tc.tile_pool(name="mmp", bufs=1, space="PSUM") as mmp,
        ):
            ident = const.tile([P, P], BF16)
            make_identity(nc, ident)

            wq_r = wpool.tile([P, KT_D, C], BF16, name="wq_r")
            wk_r = wpool.tile([P, KT_D, C], BF16, name="wk_r")
            wv_r = wpool.tile([P, KT_D, C], BF16, name="wv_r")
            wo_r = wpool.tile([P, 2, D], BF16, name="wo_r")

            # transposed inputs, one tile per (512-row chunk, kt) so the
            # per-kt DMA transposes don't serialize on tile WAW tracking
            xTpix = [[wpool.tile([P, CH], BF16, name=f"xpi{i}_{k}")
                      for k in range(KT_D)] for i in range(NCH)]
            xTpat = [[wpool.tile([P, CH], BF16, name=f"xpa{i}_{k}")
                      for k in range(KT_D)] for i in range(NCH)]
            kTc = [wpool.tile([P, 2, CH], BF16, name=f"kTc{i}")
                   for i in range(NCH)]
            qTc = [wpool.tile([P, 2, CH], BF16, name=f"qTc{i}")
                   for i in range(NCH)]
            vc = wpool.tile([P, JT, HPC, DH + 1], BF16, name="vc")
            nc.vector.memset(vc[:, :, :, DH], 1.0)
            # oT[(pair, qb)] : [(hh,dh)=128, 512 queries]
            oT = [wpool.tile([P, QB], BF16, name=f"oT{i}") for i in range(8)]

            def dma_w(dram_ap, dst, eng=None):
                (eng or nc.sync).dma_start(out=dst[:], in_=dram_ap)

            # DMA issue queues have shallow depth; spread transposes over
            # several issuing engines so transfers pipeline.
            def dmaT_span(src_dram, tiles, c0, c1, engs=None):
                engs = engs or (nc.sync,)
                i = 0
                for ch in range(c0, c1):
                    for kt in range(KT_D):
                        engs[i % len(engs)].dma_start_transpose(
                            out=tiles[ch][kt][:],
                            in_=src_dram.ap()[ch * CH:(ch + 1) * CH,
                                              kt * P:(kt + 1) * P])
                        i += 1

            # ---------------- projection helpers -------------------------
            SPANS = ((0, 496), (496, 512))

            def proj_kq_span(w_r, dstT, xT, ch, mt, a, b, alloc=None):
                pq = alloc() if alloc else mmp.tile([P, 496], F32,
                                                    name="mm")[:]
                for kt in range(KT_D):
                    nc.tensor.matmul(
                        pq[:, 0:b - a],
                        w_r[:, kt, mt * P:(mt + 1) * P],
                        xT[kt][:, a:b],
                        start=(kt == 0), stop=(kt == KT_D - 1))
                nc.vector.tensor_copy(
                    dstT[ch][:, mt, a:b], pq[:, 0:b - a])

            def proj_kq_subunits(w_r, dstT, xT, ch):
                return [
                    (lambda m=mt, aa=a, bb=b:
                     proj_kq_span(w_r, dstT, xT, ch, m, aa, bb))
                    for mt in range(2) for (a, b) in SPANS]

            # front-phase proj: ping-pong the (still idle) ring PSUM tiles
            _front = {"i": 0, "tile": None, "slot": 3}

            def _front_alloc():
                if _front["slot"] >= 3:
                    nm = "r0" if _front["i"] % 2 == 0 else "r1"
                    wid = RW if nm == "r0" else 1024
                    _front["tile"] = ringp.tile([P, wid], F32, name=nm)
                    _front["slot"] = 0
                    _front["i"] += 1
                    _front["nslots"] = 3 if nm == "r0" else 2
                sl = _front["slot"]
                _front["slot"] = sl + 1
                if _front["slot"] >= _front["nslots"]:
                    _front["slot"] = 3
                return _front["tile"][:, sl * CH:sl * CH + 496]

            def proj_v_span(ch, s):
                xT = xTpix[ch]
                pv = mmp.tile([P, 496], F32, name="mm")
                for kt in range(KT_D):
                    nc.tensor.matmul(
                        pv[:, 0:C],
                        xT[kt][:, s * P:(s + 1) * P],
                        wv_r[:, kt, :],
                        start=(kt == 0), stop=(kt == KT_D - 1))
                nc.vector.tensor_copy(
                    vc[:, ch * 4 + s, :, 0:DH],
                    pv[:, 0:C].rearrange("p (h e) -> p h e", h=HPC))

            def proj_v_subunits(ch):
                return [(lambda ss=s: proj_v_span(ch, ss)) for s in range(4)]

            # ---------------- out-projection ------------------------------
            OSPANS = ((0, 496), (496, 992), (992, 1024))

            def outproj_span(qb, qblk, ys, a, b, store, tail=False):
                if tail:
                    py = _front_alloc()
                else:
                    py = mmp.tile([P, 496], F32, name="mm")[:]
                for ct in range(2):
                    nc.tensor.matmul(
                        py[:, 0:b - a],
                        oT[ct * NQB + qb][:, qblk * P:(qblk + 1) * P],
                        wo_r[:, ct, a:b],
                        start=(ct == 0), stop=(ct == 1))
                nc.vector.tensor_copy(ys[:, a:b], py[:, 0:b - a])
                if store:
                    nc.sync.dma_start(out=yp_r[qb * QBLK + qblk],
                                      in_=ys[:])

            def outproj_subunits(qb, qblk):
                box = {}

                def mk(a, b, store):
                    def fn():
                        if "ys" not in box:
                            box["ys"] = ysp.tile([P, D], BF16, name="ys")
                        outproj_span(qb, qblk, box["ys"], a, b, store)
                    return fn
                return [mk(a, b, (a, b) == OSPANS[-1]) for (a, b) in OSPANS]

            def outproj_unit(qb, qblk):
                for fn in outproj_subunits(qb, qblk):
                    fn()

            # ================== emission ==================================
            # --- early DMAs: weights on sync, chunk-0 transposes split
            # across the scalar/vector/gpsimd issue queues (all idle now)
            dma_w(wq_v, wq_r)
            dma_w(wk_v, wk_r)
            dmaT_span(xe, xTpix, 0, 1)
            dmaT_span(pe, xTpat, 0, 1)
            dma_w(wv_v, wv_r)
            # pixel chunk 1 early (needed by g0/jt4): own queues
            dmaT_span(xe, xTpix, 1, 2)

            # --- upfront k/q projections: only the pair-0 (mt=0) columns
            # gate the first sim; pair-1 spans become group-0 fillers
            for (a, b) in SPANS:
                proj_kq_span(wk_r, kTc, xTpix[0], 0, 0, a, b)
                proj_kq_span(wq_r, qTc, xTpat[0], 0, 0, a, b)
            # (pair-1 spans of chunk 0 are scheduled as fillers)

            # --- filler schedule -----------------------------------------
            # (g, jt) -> list of sub-units enqueued there; one sub-unit is
            # popped from the queue after each sim chunk. DMA issues fire
            # immediately at their (g, jt) point.
            fillers = {}
            dma_at = {}

            def add_fillers(g, jt, fns):
                fillers.setdefault((g, jt), []).extend(fns)

            def add_dma(g, jt, fn):
                dma_at.setdefault((g, jt), []).append(fn)

            add_dma(0, 0, lambda: dmaT_span(xe, xTpix, 2, 4))
            add_dma(0, 4, lambda: dmaT_span(pe, xTpat, 1, 2))
            add_dma(0, 8, lambda: dmaT_span(pe, xTpat, 2, 4))
            add_dma(1, 6, lambda: dma_w(wo_t, wo_r))

            # explicit deadline-safe unit placement: units scheduled at
            # (g, jt) are emitted BEFORE that jt's sim chunks, so every
            # consumer (sim needs kq by its jt; attnv needs v by its jt+lag)
            # sees its producer earlier in program order.
            kc = {c: proj_kq_subunits(wk_r, kTc, xTpix[c], c)
                  for c in range(1, 4)}
            qc = {c: proj_kq_subunits(wq_r, qTc, xTpat[c], c)
                  for c in range(4)}
            vcu = {c: proj_v_subunits(c) for c in range(4)}
            kc0mt1 = [
                (lambda aa=a, bb=b: proj_kq_span(wk_r, kTc, xTpix[0], 0, 1,
                                                 aa, bb))
                for (a, b) in SPANS]

            add_fillers(0, 2, vcu[0][0:2])
            add_fillers(0, 3, vcu[0][2:4])
            add_fillers(0, 4, kc[1][0:2])
            add_fillers(0, 5, vcu[1][0:2])
            add_fillers(0, 6, vcu[1][2:4])
            add_fillers(0, 7, kc[2][0:2])
            add_fillers(0, 8, vcu[2][0:2])
            add_fillers(0, 9, vcu[2][2:4])
            add_fillers(0, 10, kc[3][0:2])
            add_fillers(0, 11, vcu[3][0:2])
            add_fillers(0, 12, vcu[3][2:4])
            add_fillers(0, 13, kc0mt1)
            add_fillers(0, 14, qc[0][2:4])     # pair-1 cols of q chunk 0
            add_fillers(0, 15, kc[1][2:4])
            add_fillers(1, 0, qc[1][0:2])
            add_fillers(1, 1, qc[1][2:4])
            add_fillers(1, 2, kc[2][2:3])
            add_fillers(1, 3, kc[2][3:4])
            add_fillers(1, 4, qc[2][0:1])
            add_fillers(1, 5, qc[2][1:2])
            add_fillers(1, 6, qc[2][2:3])
            add_fillers(1, 7, qc[2][3:4])
            add_fillers(1, 8, kc[3][2:3])
            add_fillers(1, 9, kc[3][3:4])
            add_fillers(1, 10, qc[3][0:1])
            add_fillers(1, 11, qc[3][1:2])
            add_fillers(1, 12, qc[3][2:3])
            add_fillers(1, 13, qc[3][3:4])
            # out-proj of qb i spread over groups 2i+2 / 2i+3 (qb3 at tail)
            for qbx in range(3):
                for blk in range(QBLK):
                    g = 2 * qbx + 2 + blk // 2
                    add_fillers(g, 4 + 7 * (blk % 2),
                                outproj_subunits(qbx, blk))

            # --- attention groups ----------------------------------------
            # group order: (pair0,qb0),(pair1,qb0),(pair0,qb1),...
            groups = [(pair, qb) for qb in range(NQB) for pair in range(2)]

            state = {"c": 0, "rt": None, "et": None, "tile_chunks": [],
                     "prev_chunks": []}
            pending = deque()      # chunks with exp emitted? no: awaiting
            attnv_count = [0] * 8  # per-group emitted attnv chunks
            po_tiles = [None] * 8

            def normalize_group(g):
                pair, qb = groups[g]
                po0, po1 = po_tiles[g]
                rz = rzp.tile([P, 8], F32, name="rz")
                nc.vector.reciprocal(
                    rz[:, 0:4], po0[:, :, DH:DH + 1].rearrange(
                        "p a c -> p (a c)"))
                nc.vector.reciprocal(
                    rz[:, 4:8], po1[:, :, DH:DH + 1].rearrange(
                        "p a c -> p (a c)"))
                obD = obp.tile([P, 2, 2, DH], BF16, name="obD")
                obP = obp.tile([P, 2, 2, DH], BF16, name="obP")
                for hh in range(2):
                    po_h = po_tiles[g][hh]
                    for blk in range(QBLK):
                        ob = obD if blk % 2 == 0 else obP
                        nc.vector.tensor_scalar_mul(
                            ob[:, blk // 2, hh, :],
                            po_h[:, blk, 0:DH],
                            rz[:, hh * 4 + blk:hh * 4 + blk + 1])
                # transpose back to [(hh,dh), q] for the out-projection
                pt = pop.tile([P, QBLK, P], BF16, name="po0")
                for blk in range(QBLK):
                    ob = obD if blk % 2 == 0 else obP
                    nc.tensor.transpose(
                        pt[:, blk, :],
                        ob[:, blk // 2, :, :].rearrange("p a b -> p (a b)"),
                        ident[:])
                dst = oT[pair * NQB + qb]
                for blk in range(QBLK):
                    nc.vector.tensor_copy(dst[:, blk * P:(blk + 1) * P],
                                          pt[:, blk, :])

            def emit_attnv(g, jt, hh, et, off):
                pair, qb = groups[g]
                if attnv_count[g] == 0:
                    po_tiles[g] = (pop.tile([P, QBLK, DH + 1], F32,
                                            name="po0"),
                                   pop.tile([P, QBLK, DH + 1], F32,
                                            name="po1"))
                po = po_tiles[g][hh]
                h = pair * 2 + hh
                # one accumulation group per po tile (2KB psum zero region):
                # start zeroes the whole region; later blk slices land on
                # pending-zero bytes, so only the very first matmul starts
                # and only the very last stops.
                for blk in range(QBLK):
                    nc.tensor.matmul(
                        po[:, blk, :],
                        et[:, off + blk * P:off + (blk + 1) * P],
                        vc[:, jt, h, :],
                        start=(jt == 0 and blk == 0),
                        stop=(jt == JT - 1 and blk == QBLK - 1))
                attnv_count[g] += 1
                if attnv_count[g] == 2 * JT:
                    normalize_group(g)

            # ring: 5-chunk cycle -> r0 slots 0..2 (1536 wide), r1 slots
            # 0..1 (1024 wide).
            def emit_chunk(g, jt, hh):
                c = state["c"]
                cyc = c % 5
                if cyc in (0, 3):
                    wid = RW if cyc == 0 else 1024
                    state["rt"] = ringp.tile([P, wid], F32,
                                             name="r0" if cyc == 0 else "r1")
                    state["et"] = eTp.tile([P, wid], BF16, name="eT")
                    state["tile_chunks"] = []
                slot = cyc if cyc < 3 else cyc - 3
                last = cyc in (2, 4)
                pair, qb = groups[g]
                rt, et = state["rt"], state["et"]
                nc.tensor.matmul(
                    rt[:, slot * CH:(slot + 1) * CH],
                    kTc[jt // 4][hh * DH:(hh + 1) * DH, pair,
                                 (jt % 4) * P:(jt % 4 + 1) * P],
                    qTc[qb][hh * DH:(hh + 1) * DH, pair, :],
                    start=True, stop=True)
                state["tile_chunks"].append((g, jt, hh, et, slot * CH))
                state["c"] = c + 1
                if last:
                    nc.scalar.activation(et[:], rt[:], EXP, scale=SCALE)
                    # drain attnv for the PREVIOUS tile (one-tile lag so
                    # the PE never parks on an in-flight exp)
                    for (gg, jj, hhh, ee, off) in state["prev_chunks"]:
                        emit_attnv(gg, jj, hhh, ee, off)
                    state["prev_chunks"] = state["tile_chunks"]
                    state["tile_chunks"] = []

            for g in range(8):
                for jt in range(JT):
                    for fn in dma_at.get((g, jt), ()):
                        fn()
                    for fn in fillers.get((g, jt), ()):
                        fn()
                    for hh in range(2):
                        emit_chunk(g, jt, hh)

            # flush the final partially-filled ring tile + lagged drains
            if state["tile_chunks"]:
                nchunks = len(state["tile_chunks"])
                rt, et = state["rt"], state["et"]
                nc.scalar.activation(et[:, 0:nchunks * CH],
                                     rt[:, 0:nchunks * CH], EXP, scale=SCALE)
            for (gg, jj, hhh, ee, off) in state["prev_chunks"]:
                emit_attnv(gg, jj, hhh, ee, off)
            for (gg, jj, hhh, ee, off) in state["tile_chunks"]:
                emit_attnv(gg, jj, hhh, ee, off)

            # tail out-projection for qb3 through the now-free ring psum
            _front["slot"] = 3
            for blk in range(QBLK):
                ys = ysp.tile([P, D], BF16, name="ys")
                for (a, b) in OSPANS:
                    outproj_span(3, blk, ys, a, b, (a, b) == OSPANS[-1],
                                 tail=True)

    nc.compile()
    return nc


def get_nc():
    if "nc" not in _cache:
        _cache["nc"] = _build_nc()
    return _cache["nc"]


def make_core_inputs(patch_embed, pixel_embed, Wq, Wk, Wv, Wo, c):
    import ml_dtypes

    bf16 = ml_dtypes.bfloat16
    b, g = divmod(c, HPC)
    sl = slice(g * C, (g + 1) * C)
    return {
        "pe": np.ascontiguousarray(np.asarray(patch_embed[b]).astype(bf16)),
        "xe": np.ascontiguousarray(np.asarray(pixel_embed[b]).astype(bf16)),
        "wq": np.ascontiguousarray(np.asarray(Wq[:, sl]).astype(bf16)),
        "wk": np.ascontiguousarray(np.asarray(Wk[:, sl]).astype(bf16)),
        "wv": np.ascontiguousarray(np.asarray(Wv[:, sl]).astype(bf16)),
        "wo": np.ascontiguousarray(np.asarray(Wo[sl, :]).astype(bf16)),
    }


def kernel(patch_embed, pixel_embed, Wq, Wk, Wv, Wo, bo):
    from concourse.bass_utils import run_bass_kernel_spmd

    nc = get_nc()
    in_maps = [make_core_inputs(patch_embed, pixel_embed, Wq, Wk, Wv, Wo, c)
               for c in range(N_CORES)]
    res = run_bass_kernel_spmd(nc, in_maps, core_ids=list(range(N_CORES)))
    out = np.empty((B, N, D), dtype=np.float32)
    for b in range(B):
        acc = res.results[b * HPC + 0]["yp"].astype(np.float32)
        for g in range(1, HPC):
            acc = acc + res.results[b * HPC + g]["yp"].astype(np.float32)
        out[b] = acc + np.asarray(bo, dtype=np.float32)[None, :]
    return out


# revision 33
# speedup vs baseline: 1.2012x; 1.0106x over previous
"""Cross-attention kernel for Trainium2, 8-core tensor-parallel over heads.

Problem (fixed shapes, fp32):
    patch_embed [2, 2048, 1024], pixel_embed [2, 2048, 1024]
    Wq/Wk/Wv [1024, 1024], Wo [1024, 1024], bo [1024]
    16 heads x 64 dim_head, softmax cross-attention, out [2, 2048, 1024].

Sharding: core c handles batch b = c // 4 and head-group g = c % 4
(4 heads = 256 inner cols). Each core computes a partial output
(its heads' contribution to out @ Wo) in bf16; host sums the 4 partials
per batch and adds the bias.

Per-core device program (all matmuls bf16, fp32 PSUM):
  - inputs land TRANSPOSED in SBUF via XBAR DMA-transpose (no PE/DVE
    transpose cost), except patch chunk 0 which is loaded natural and
    PE-transposed to cut the pipeline-fill latency.
  - kT/qT = W^T @ xT  [ (hh,dh)=128 part, pair, seq ];  v natural.
  - attention runs in (pair, qb=512-query) groups over 16 key tiles;
    logits stream through a ring of two [128,1536] fp32 PSUM tiles so
    every ACT exp instruction is 1536 wide (amortizes ACT fixed cost).
  - attn@v uses out[q,dh] orientation: lhsT = exp-tile slice [128k,128q],
    rhs = v_aug [128k, 65] (65th col = ones -> softmax denominator Z),
    so each matmul streams only 65 free elements instead of 512.
  - normalization is a per-partition reciprocal + tensor_scalar_mul
    (Z sits on the free axis), split across DVE and Pool engines.
  - normalized o [q, (hh,dh)] is PE-transposed back to [(hh,dh), q]
    for the output projection; projections/out-proj are interleaved
    into the ACT-bound attention phase as filler work on a static
    schedule so the PE never idles.
"""

import numpy as np
from collections import deque

HEADS = 16
DH = 64
B = 2
N = 2048          # query seq len
M = 2048          # key seq len
D = 1024
N_CORES = 8
HPC = 4           # heads per core
C = HPC * DH      # 256 inner cols per core
SCALE = DH ** -0.5
P = 128
CH = 512          # input chunk span (seq rows)
NCH = N // CH     # 4 chunks
KT_D = D // P     # 8 contraction tiles for projections
JT = M // P       # 16 key tiles
RW = 1536         # exp ring width (3 x 512 chunks)
QB = 512          # queries per attention group
NQB = N // QB     # 4 query groups
QBLK = QB // P    # 4 128-query blocks per group

_cache = {}


def _build_nc():
    import concourse.bacc as bacc
    import concourse.mybir as mybir
    import concourse.tile as tile
    from concourse.masks import make_identity

    F32 = mybir.dt.float32
    BF16 = mybir.dt.bfloat16
    EXP = mybir.ActivationFunctionType.Exp

    nc = bacc.Bacc("TRN2", target_bir_lowering=False, debug=False,
                   num_devices=N_CORES)

    pe = nc.dram_tensor("pe", [N, D], BF16, kind="ExternalInput")
    xe = nc.dram_tensor("xe", [M, D], BF16, kind="ExternalInput")
    wq = nc.dram_tensor("wq", [D, C], BF16, kind="ExternalInput")
    wk = nc.dram_tensor("wk", [D, C], BF16, kind="ExternalInput")
    wv = nc.dram_tensor("wv", [D, C], BF16, kind="ExternalInput")
    wo = nc.dram_tensor("wo", [C, D], BF16, kind="ExternalInput")
    yp = nc.dram_tensor("yp", [N, D], BF16, kind="ExternalOutput")

    pe_nat = pe.ap().rearrange("(ch s p) d -> ch p s d", p=P, s=4)
    wq_v, wk_v, wv_v = (w.ap().rearrange("(ko ki) c -> ki ko c", ki=P)
                        for w in (wq, wk, wv))
    wo_t = wo.ap().rearrange("(ko ki) n -> ki ko n", ki=P)
    yp_r = yp.ap().rearrange("(qq p) d -> qq p d", p=P)   # 16 x [128,1024]

    with tile.TileContext(nc) as tc:
        with (
            tc.tile_pool(name="const", bufs=1) as const,
            tc.tile_pool(name="wpool", bufs=1) as wpool,
            tc.tile_pool(name="natp", bufs=1) as natp,
            tc.tile_pool(name="eTp", bufs=6) as eTp,
            tc.tile_pool(name="obp", bufs=3) as obp,
            tc.tile_pool(name="rzp", bufs=3) as rzp,
            tc.tile_pool(name="ysp", bufs=4) as ysp,
            tc.tile_pool(name="ring", bufs=1, space="PSUM") as ringp,
            tc.tile_pool(name="pop", bufs=1, space="PSUM") as pop,
            tc.tile_pool(name="mmp", bufs=1, space="PSUM") as mmp,
        ):
            ident = const.tile([P, P], BF16)
            make_identity(nc, ident)

            wq_r = wpool.tile([P, KT_D, C], BF16, name="wq_r")
            wk_r = wpool.tile([P, KT_D, C], BF16, name="wk_r")
            wv_r = wpool.tile([P, KT_D, C], BF16, name="wv_r")
            wo_r = wpool.tile([P, 2, D], BF16, name="wo_r")

            # transposed inputs, one tile per (512-row chunk, kt) so the
            # per-kt DMA transposes don't serialize on tile WAW tracking
            xTpix = [[wpool.tile([P, CH], BF16, name=f"xpi{i}_{k}")
                      for k in range(KT_D)] for i in range(NCH)]
            xTpat = [[wpool.tile([P, CH], BF16, name=f"xpa{i}_{k}")
                      for k in range(KT_D)] for i in range(NCH)]
            kTc = [wpool.tile([P, 2, CH], BF16, name=f"kTc{i}")
                   for i in range(NCH)]
            qTc = [wpool.tile([P, 2, CH], BF16, name=f"qTc{i}")
                   for i in range(NCH)]
            vc = wpool.tile([P, JT, HPC, DH + 1], BF16, name="vc")
            nc.vector.memset(vc[:, :, :, DH], 1.0)
            # oT[(pair, qb)] : [(hh,dh)=128, 512 queries]
            oT = [wpool.tile([P, QB], BF16, name=f"oT{i}") for i in range(8)]

            def dma_w(dram_ap, dst, eng=None):
                (eng or nc.sync).dma_start(out=dst[:], in_=dram_ap)

            # DMA issue queues have shallow depth; spread transposes over
            # several issuing engines so transfers pipeline.
            def dmaT_span(src_dram, tiles, c0, c1, engs=None):
                engs = engs or (nc.sync,)
                i = 0
                for ch in range(c0, c1):
                    for kt in range(KT_D):
                        engs[i % len(engs)].dma_start_transpose(
                            out=tiles[ch][kt][:],
                            in_=src_dram.ap()[ch * CH:(ch + 1) * CH,
                                              kt * P:(kt + 1) * P])
                        i += 1

            # ---------------- projection helpers -------------------------
            SPANS = ((0, 496), (496, 512))

            def proj_kq_span(w_r, dstT, xT, ch, mt, a, b, alloc=None):
                pq = alloc() if alloc else mmp.tile([P, 496], F32,
                                                    name="mm")[:]
                for kt in range(KT_D):
                    nc.tensor.matmul(
                        pq[:, 0:b - a],
                        w_r[:, kt, mt * P:(mt + 1) * P],
                        xT[kt][:, a:b],
                        start=(kt == 0), stop=(kt == KT_D - 1))
                nc.vector.tensor_copy(
                    dstT[ch][:, mt, a:b], pq[:, 0:b - a])

            def proj_kq_subunits(w_r, dstT, xT, ch):
                return [
                    (lambda m=mt, aa=a, bb=b:
                     proj_kq_span(w_r, dstT, xT, ch, m, aa, bb))
                    for mt in range(2) for (a, b) in SPANS]

            # front-phase proj: ping-pong the (still idle) ring PSUM tiles
            _front = {"i": 0, "tile": None, "slot": 3}

            def _front_alloc():
                if _front["slot"] >= 3:
                    nm = "r0" if _front["i"] % 2 == 0 else "r1"
                    wid = RW if nm == "r0" else 1024
                    _front["tile"] = ringp.tile([P, wid], F32, name=nm)
                    _front["slot"] = 0
                    _front["i"] += 1
                    _front["nslots"] = 3 if nm == "r0" else 2
                sl = _front["slot"]
                _front["slot"] = sl + 1
                if _front["slot"] >= _front["nslots"]:
                    _front["slot"] = 3
                return _front["tile"][:, sl * CH:sl * CH + 496]

            def proj_v_span(ch, s):
                xT = xTpix[ch]
                pv = mmp.tile([P, 496], F32, name="mm")
                for kt in range(KT_D):
                    nc.tensor.matmul(
                        pv[:, 0:C],
                        xT[kt][:, s * P:(s + 1) * P],
                        wv_r[:, kt, :],
                        start=(kt == 0), stop=(kt == KT_D - 1))
                nc.vector.tensor_copy(
                    vc[:, ch * 4 + s, :, 0:DH],
                    pv[:, 0:C].rearrange("p (h e) -> p h e", h=HPC))

            def proj_v_subunits(ch):
                return [(lambda ss=s: proj_v_span(ch, ss)) for s in range(4)]

            # ---------------- out-projection ------------------------------
            OSPANS = ((0, 496), (496, 992), (992, 1024))

            def outproj_span(qb, qblk, ys, a, b, store, tail=False):
                if tail:
                    py = _front_alloc()
                else:
                    py = mmp.tile([P, 496], F32, name="mm")[:]
                for ct in range(2):
                    nc.tensor.matmul(
                        py[:, 0:b - a],
                        oT[ct * NQB + qb][:, qblk * P:(qblk + 1) * P],
                        wo_r[:, ct, a:b],
                        start=(ct == 0), stop=(ct == 1))
                nc.vector.tensor_copy(ys[:, a:b], py[:, 0:b - a])
                if store:
                    nc.sync.dma_start(out=yp_r[qb * QBLK + qblk],
                                      in_=ys[:])

            def outproj_subunits(qb, qblk):
                box = {}

                def mk(a, b, store):
                    def fn():
                        if "ys" not in box:
                            box["ys"] = ysp.tile([P, D], BF16, name="ys")
                        outproj_span(qb, qblk, box["ys"], a, b, store)
                    return fn
                return [mk(a, b, (a, b) == OSPANS[-1]) for (a, b) in OSPANS]

            def outproj_unit(qb, qblk):
                for fn in outproj_subunits(qb, qblk):
                    fn()

            # ================== emission ==================================
            # --- early DMAs: weights on sync, chunk-0 transposes split
            # across the scalar/vector/gpsimd issue queues (all idle now)
            dma_w(wq_v, wq_r)
            dma_w(wk_v, wk_r)
            dmaT_span(xe, xTpix, 0, 1)
            dmaT_span(pe, xTpat, 0, 1)
            dma_w(wv_v, wv_r)
            # pixel chunk 1 early (needed by g0/jt4): own queues
            dmaT_span(xe, xTpix, 1, 2)

            # --- upfront k/q projections: only the pair-0 (mt=0) columns
            # gate the first sim; pair-1 spans become group-0 fillers
            for (a, b) in SPANS:
                proj_kq_span(wk_r, kTc, xTpix[0], 0, 0, a, b)
                proj_kq_span(wq_r, qTc, xTpat[0], 0, 0, a, b)
            # (pair-1 spans of chunk 0 are scheduled as fillers)

            # --- filler schedule -----------------------------------------
            # (g, jt) -> list of sub-units enqueued there; one sub-unit is
            # popped from the queue after each sim chunk. DMA issues fire
            # immediately at their (g, jt) point.
            fillers = {}
            dma_at = {}

            def add_fillers(g, jt, fns):
                fillers.setdefault((g, jt), []).extend(fns)

            def add_dma(g, jt, fn):
                dma_at.setdefault((g, jt), []).append(fn)

            add_dma(0, 0, lambda: dmaT_span(xe, xTpix, 2, 4))
            add_dma(0, 4, lambda: dmaT_span(pe, xTpat, 1, 2))
            add_dma(0, 8, lambda: dmaT_span(pe, xTpat, 2, 4))
            add_dma(1, 6, lambda: dma_w(wo_t, wo_r))

            # explicit deadline-safe unit placement: units scheduled at
            # (g, jt) are emitted BEFORE that jt's sim chunks, so every
            # consumer (sim needs kq by its jt; attnv needs v by its jt+lag)
            # sees its producer earlier in program order.
            kc = {c: proj_kq_subunits(wk_r, kTc, xTpix[c], c)
                  for c in range(1, 4)}
            qc = {c: proj_kq_subunits(wq_r, qTc, xTpat[c], c)
                  for c in range(4)}
            vcu = {c: proj_v_subunits(c) for c in range(4)}
            kc0mt1 = [
                (lambda aa=a, bb=b: proj_kq_span(wk_r, kTc, xTpix[0], 0, 1,
                                                 aa, bb))
                for (a, b) in SPANS]

            add_fillers(0, 2, vcu[0][0:2])
            add_fillers(0, 3, vcu[0][2:4])
            add_fillers(0, 4, kc[1][0:2])
            add_fillers(0, 5, vcu[1][0:2])
            add_fillers(0, 6, vcu[1][2:4])
            add_fillers(0, 7, kc[2][0:2])
            add_fillers(0, 8, vcu[2][0:2])
            add_fillers(0, 9, vcu[2][2:4])
            add_fillers(0, 10, kc[3][0:2])
            add_fillers(0, 11, vcu[3][0:2])
            add_fillers(0, 12, vcu[3][2:4])
            add_fillers(0, 13, kc0mt1)
            add_fillers(0, 14, qc[0][2:4])     # pair-1 cols of q chunk 0
            add_fillers(0, 15, kc[1][2:4])
            add_fillers(1, 0, qc[1][0:2])
            add_fillers(1, 1, qc[1][2:4])
            add_fillers(1, 2, kc[2][2:3])
            add_fillers(1, 3, kc[2][3:4])
            add_fillers(1, 4, qc[2][0:1])
            add_fillers(1, 5, qc[2][1:2])
            add_fillers(1, 6, qc[2][2:3])
            add_fillers(1, 7, qc[2][3:4])
            add_fillers(1, 8, kc[3][2:3])
            add_fillers(1, 9, kc[3][3:4])
            add_fillers(1, 10, qc[3][0:1])
            add_fillers(1, 11, qc[3][1:2])
            add_fillers(1, 12, qc[3][2:3])
            add_fillers(1, 13, qc[3][3:4])
            # out-proj of qb i spread over groups 2i+2 / 2i+3 (qb3 at tail)
            for qbx in range(3):
                for blk in range(QBLK):
                    g = 2 * qbx + 2 + blk // 2
                    add_fillers(g, 4 + 7 * (blk % 2),
                                outproj_subunits(qbx, blk))

            # --- attention groups ----------------------------------------
            # group order: (pair0,qb0),(pair1,qb0),(pair0,qb1),...
            groups = [(pair, qb) for qb in range(NQB) for pair in range(2)]

            state = {"c": 0, "rt": None, "et": None, "tile_chunks": [],
                     "prev_chunks": []}
            pending = deque()      # chunks with exp emitted? no: awaiting
            attnv_count = [0] * 8  # per-group emitted attnv chunks
            po_tiles = [None] * 8

            def normalize_group(g):
                pair, qb = groups[g]
                po0, po1 = po_tiles[g]
                rz = rzp.tile([P, 8], F32, name="rz")
                nc.vector.reciprocal(
                    rz[:, 0:4], po0[:, :, DH:DH + 1].rearrange(
                        "p a c -> p (a c)"))
                nc.vector.reciprocal(
                    rz[:, 4:8], po1[:, :, DH:DH + 1].rearrange(
                        "p a c -> p (a c)"))
                obD = obp.tile([P, 2, 2, DH], BF16, name="obD")
                obP = obp.tile([P, 2, 2, DH], BF16, name="obP")
                for hh in range(2):
                    po_h = po_tiles[g][hh]
                    for blk in range(QBLK):
                        ob = obD if blk % 2 == 0 else obP
                        nc.vector.tensor_scalar_mul(
                            ob[:, blk // 2, hh, :],
                            po_h[:, blk, 0:DH],
                            rz[:, hh * 4 + blk:hh * 4 + blk + 1])
                # transpose back to [(hh,dh), q] for the out-projection
                pt = pop.tile([P, QBLK, P], BF16, name="po0")
                for blk in range(QBLK):
                    ob = obD if blk % 2 == 0 else obP
                    nc.tensor.transpose(
                        pt[:, blk, :],
                        ob[:, blk // 2, :, :].rearrange("p a b -> p (a b)"),
                        ident[:])
                dst = oT[pair * NQB + qb]
                for blk in range(QBLK):
                    nc.vector.tensor_copy(dst[:, blk * P:(blk + 1) * P],
                                          pt[:, blk, :])

            def emit_attnv(g, jt, hh, et, off):
                pair, qb = groups[g]
                if attnv_count[g] == 0:
                    po_tiles[g] = (pop.tile([P, QBLK, DH + 1], F32,
                                            name="po0"),
                                   pop.tile([P, QBLK, DH + 1], F32,
                                            name="po1"))
                po = po_tiles[g][hh]
                h = pair * 2 + hh
                # one accumulation group per po tile (2KB psum zero region):
                # start zeroes the whole region; later blk slices land on
                # pending-zero bytes, so only the very first matmul starts
                # and only the very last stops.
                for blk in range(QBLK):
                    nc.tensor.matmul(
                        po[:, blk, :],
                        et[:, off + blk * P:off + (blk + 1) * P],
                        vc[:, jt, h, :],
                        start=(jt == 0 and blk == 0),
                        stop=(jt == JT - 1 and blk == QBLK - 1))
                attnv_count[g] += 1
                if attnv_count[g] == 2 * JT:
                    normalize_group(g)

            # ring: 5-chunk cycle -> r0 slots 0..2 (1536 wide), r1 slots
            # 0..1 (1024 wide).
            def emit_chunk(g, jt, hh):
                c = state["c"]
                cyc = c % 5
                if cyc in (0, 3):
                    wid = RW if cyc == 0 else 1024
                    state["rt"] = ringp.tile([P, wid], F32,
                                             name="r0" if cyc == 0 else "r1")
                    state["et"] = eTp.tile([P, wid], BF16, name="eT")
                    state["tile_chunks"] = []
                slot = cyc if cyc < 3 else cyc - 3
                last = cyc in (2, 4)
                pair, qb = groups[g]
                rt, et = state["rt"], state["et"]
                nc.tensor.matmul(
                    rt[:, slot * CH:(slot + 1) * CH],
                    kTc[jt // 4][hh * DH:(hh + 1) * DH, pair,
                                 (jt % 4) * P:(jt % 4 + 1) * P],
                    qTc[qb][hh * DH:(hh + 1) * DH, pair, :],
                    start=True, stop=True)
                state["tile_chunks"].append((g, jt, hh, et, slot * CH))
                state["c"] = c + 1
                if last:
                    nc.scalar.activation(et[:], rt[:], EXP, scale=SCALE)
                    # drain attnv for the PREVIOUS tile (one-tile lag so
                    # the PE never parks on an in-flight exp)
                    for (gg, jj, hhh, ee, off) in state["prev_chunks"]:
                        emit_attnv(gg, jj, hhh, ee, off)
                    state["prev_chunks"] = state["tile_chunks"]
                    state["tile_chunks"] = []

            for g in range(8):
                for jt in range(JT):
                    for fn in dma_at.get((g, jt), ()):
                        fn()
                    for fn in fillers.get((g, jt), ()):
                        fn()
                    for hh in range(2):
                        emit_chunk(g, jt, hh)

            # flush the final partially-filled ring tile + lagged drains
            if state["tile_chunks"]:
                nchunks = len(state["tile_chunks"])
                rt, et = state["rt"], state["et"]
                nc.scalar.activation(et[:, 0:nchunks * CH],
                                     rt[:, 0:nchunks * CH], EXP, scale=SCALE)
            for (gg, jj, hhh, ee, off) in state["prev_chunks"]:
                emit_attnv(gg, jj, hhh, ee, off)
            for (gg, jj, hhh, ee, off) in state["tile_chunks"]:
                emit_attnv(gg, jj, hhh, ee, off)

            # tail out-projection for qb3 through the now-free ring psum
            _front["slot"] = 3
            for blk in range(QBLK):
                ys = ysp.tile([P, D], BF16, name="ys")
                for (a, b) in OSPANS:
                    outproj_span(3, blk, ys, a, b, (a, b) == OSPANS[-1],
                                 tail=True)

    nc.compile()
    return nc


def get_nc():
    if "nc" not in _cache:
        _cache["nc"] = _build_nc()
    return _cache["nc"]


def make_core_inputs(patch_embed, pixel_embed, Wq, Wk, Wv, Wo, c):
    import ml_dtypes

    bf16 = ml_dtypes.bfloat16
    b, g = divmod(c, HPC)
    sl = slice(g * C, (g + 1) * C)
    return {
        "pe": np.ascontiguousarray(np.asarray(patch_embed[b]).astype(bf16)),
        "xe": np.ascontiguousarray(np.asarray(pixel_embed[b]).astype(bf16)),
        "wq": np.ascontiguousarray(np.asarray(Wq[:, sl]).astype(bf16)),
        "wk": np.ascontiguousarray(np.asarray(Wk[:, sl]).astype(bf16)),
        "wv": np.ascontiguousarray(np.asarray(Wv[:, sl]).astype(bf16)),
        "wo": np.ascontiguousarray(np.asarray(Wo[sl, :]).astype(bf16)),
    }


def kernel(patch_embed, pixel_embed, Wq, Wk, Wv, Wo, bo):
    from concourse.bass_utils import run_bass_kernel_spmd

    nc = get_nc()
    in_maps = [make_core_inputs(patch_embed, pixel_embed, Wq, Wk, Wv, Wo, c)
               for c in range(N_CORES)]
    res = run_bass_kernel_spmd(nc, in_maps, core_ids=list(range(N_CORES)))
    out = np.empty((B, N, D), dtype=np.float32)
    for b in range(B):
        acc = res.results[b * HPC + 0]["yp"].astype(np.float32)
        for g in range(1, HPC):
            acc = acc + res.results[b * HPC + g]["yp"].astype(np.float32)
        out[b] = acc + np.asarray(bo, dtype=np.float32)[None, :]
    return out
